# revision 48
# baseline (speedup 1.0000x reference)
"""Trainium2 Bass kernel for a dense transformer block (RMSNorm -> QKV+RoPE ->
attention -> proj -> RMSNorm -> SiLU FFN), sharded over 8 NeuronCores.

The dominant cost in this environment is host<->device transfer over the
axon tunnel (~50-90 MB/s, ~100 ms latency), so the design minimizes shipped
bytes and transfer count:

- Host ships ONE packed blob per core (~2.1 MB): the core's own 512-token
  slice of x = z_H + z_L (int8 with per-token bf16 scales), 1/8 row-shards
  of each weight matrix (int8 with per-row bf16 scales, norm gains folded
  in), a 1/8 shard of the RoPE table, the core's own-query RoPE rows, and
  a per-core attention bias row. No host-side transposes.
- On device, the blobs are AllGathered (fast on-chip links; bf16 header
  and int8 section gathered separately so every read is dep-tracked),
  giving every core all 4096 tokens of x and the full weights. Weights
  cross the tunnel once (12 MB int8) instead of 8x (192 MB fp32).
- Each core computes K/V for all 4096 tokens (both batches) and Q for its
  own 512 tokens, then attends over all 4096 keys with a -30 pre-softmax
  bias masking other-batch keys. The bias is shipped as data, so the
  device program is rank-free (pure SPMD, no partition-id).
- proj/norm2/FFN run on the core's own 512 tokens with full weights.
  Output is the core's [512, D] slice, int8-quantized per token with the
  bf16 scale packed into the last two bytes of each row.
- jax's persistent compilation cache is enabled: without it every
  run_bass_kernel_spmd call re-runs the walrus NEFF build (~1 s).

Tunnel traffic per call: ~17 MB in + 4 MB zero-donation + 4 MB out,
vs ~340 MB for the replicate-everything baseline.
"""

import math
from contextlib import ExitStack

import ml_dtypes
import numpy as np

import jax as _jax

# Cache compiled PJRT executables on disk: without this, every
# run_bass_kernel_spmd call re-runs the walrus NEFF build (~1s) because the
# fresh jit wrapper defeats jax's in-memory caches.
try:
    _jax.config.update("jax_compilation_cache_dir", "/tmp/jaxcache")
    _jax.config.update("jax_persistent_cache_min_compile_time_secs", 0.0)
    _jax.config.update("jax_persistent_cache_min_entry_size_bytes", 0)
except Exception:
    pass

import concourse.bass as bass
from concourse import bacc
import concourse.mybir as mybir
import concourse.tile as tile
from concourse.bass_utils import run_bass_kernel_spmd
from concourse.masks import make_identity

FP32 = mybir.dt.float32
BF16 = mybir.dt.bfloat16
I8 = mybir.dt.int8
AF = mybir.ActivationFunctionType
ALU = mybir.AluOpType

B, S, D, F, H, DH = 2, 2048, 1024, 4096, 16, 64
HALF = DH // 2
NCORES = 8
CPB = NCORES // B       # cores per batch
QN = S // CPB           # own query tokens per core (512)
T = B * S               # gathered tokens across all cores (4096)
EPS = 1e-6
ROPE_BASE = 10000.0
P = 128
W = 512                 # matmul moving-dim window
HPW = W // DH           # heads per window (8)
QW = 512                # attention query window
NQW = QN // QW
KD = D // P             # 8
KF = F // P             # 32
TT = T // P             # 32 gathered token tiles
QT = QN // P            # 4
MASK_BIAS = -30.0

FR = F // NCORES                        # wf2 shard rows (512)

# blob layout: bf16 header section, then int8 section (x + weights).
# bf16-element offsets:
SR = S // NCORES                        # cos|sin rows shipped per core (256)
OCS = 0                                 # cos|sin table shard [SR, 2*HALF] bf16
OCSQ = OCS + SR * 2 * HALF              # own-query cos|sin [QN, 2*HALF] bf16
OBIAS = OCSQ + QN * 2 * HALF            # key bias row [T] bf16
OSALL = OBIAS + T                       # ALL weight row scales, replicated:
                                        #   [D wqkv | D proj | D f1 | F f2] bf16
OSX = OSALL + 3 * D + F                 # own-token x scales [QN] bf16
OSXA = OSX + QN                         # all-token x scales [T] bf16
OW8 = OSXA + T                          # end of bf16 section (bf16 elems)
# offsets within the int8 section (int8 elements == bytes):
R8X = 0                                 # x_own [QN, D] int8
R8QKV = R8X + QN * D                    # [P, 3D] int8
R8PROJ = R8QKV + P * 3 * D              # [P, D] int8
R8F1 = R8PROJ + P * D                   # [P, F] int8
R8F2 = R8F1 + P * F                     # [FR, D] int8
W8SIZE = R8F2 + FR * D
BW = 2 * OW8                            # byte offset of int8 section in blob
BLOB_BYTES = BW + W8SIZE
BLOB = BLOB_BYTES // 2                  # bf16 elements


def build_bass():
    """Emit the per-core program. All cores run this same NEFF."""
    nc = bacc.Bacc()
    blob = nc.dram_tensor("blob", [BLOB], BF16, kind="ExternalInput")
    outd = nc.dram_tensor("outt", [QN, D + 2], I8, kind="ExternalOutput")

    with tile.TileContext(nc) as tc:
        with ExitStack() as ctx:
            pool = lambda name, bufs, **kw: ctx.enter_context(
                tc.tile_pool(name=name, bufs=bufs, **kw)
            )
            dram = pool("dram", 1, space="DRAM")
            bounce = dram.tile([OW8], BF16, tag="bounce")
            bounce8 = dram.tile([W8SIZE], I8, tag="bounce8")
            gath = dram.tile([NCORES * OW8], BF16, tag="gath")
            gath8 = dram.tile([NCORES * W8SIZE], I8, tag="gath8")
            bap = blob[:]
            bap8 = bap.bitcast(I8)
            nc.gpsimd.dma_start(bounce, blob[0:OW8])
            nc.gpsimd.dma_start(
                bounce8,
                bass.AP(tensor=bap8.tensor, offset=bap8.offset + BW, ap=[[1, W8SIZE]]),
            )
            nc.gpsimd.collective_compute(
                "AllGather",
                ALU.bypass,
                replica_groups=[list(range(NCORES))],
                ins=[bounce.opt()],
                outs=[gath.opt()],
            )
            nc.gpsimd.collective_compute(
                "AllGather",
                ALU.bypass,
                replica_groups=[list(range(NCORES))],
                ins=[bounce8.opt()],
                outs=[gath8.opt()],
            )
            gap = gath[:]
            gap8 = gath8[:]

            def gv(off, dims):
                return bass.AP(
                    tensor=gap.tensor, offset=gap.offset + off,
                    ap=[list(d) for d in dims],
                )

            def gv8(off_bytes, dims):
                return bass.AP(
                    tensor=gap8.tensor, offset=gap8.offset + off_bytes,
                    ap=[list(d) for d in dims],
                )

            def bv(off, dims):
                return bass.AP(
                    tensor=bap.tensor, offset=bap.offset + off,
                    ap=[list(d) for d in dims],
                )

            def bv8(off_bytes, dims):
                # blob is an ExternalInput (written before kernel start), so
                # the untracked bitcast view is race-free
                return bass.AP(
                    tensor=bap8.tensor, offset=bap8.offset + BW + off_bytes,
                    ap=[list(d) for d in dims],
                )

            # ---- persistent small tiles ----
            psingle = pool("psingle", 1)
            ident = psingle.tile([P, P], BF16)
            make_identity(nc, ident)
            ones_col = psingle.tile([P, 1], BF16)
            nc.vector.memset(ones_col, 1.0)
            ones_row = psingle.tile([1, P], FP32)
            nc.vector.memset(ones_row, 1.0)
            eps_t = psingle.tile([P, 1], FP32)
            nc.vector.memset(eps_t, EPS)
            zero_t = psingle.tile([P, 1], FP32)
            nc.vector.memset(zero_t, 0.0)

            pqT = pool("pqT", 1)
            qT = pqT.tile([P, KD, QN], BF16, tag="qT")        # roped q, [dh, hc, tok]
            pattn = pool("pattn", 1)
            attn = pattn.tile([P, KD, QN], BF16, tag="attn")  # attn out, [dh, hc, tok]
            pxres = pool("pxres", 1)
            xres = pxres.tile([P, KD, QN], FP32, tag="xres")  # own x -> residual accum
            pbias = pool("pbias", 1)
            bias_f = pbias.tile([P, TT], FP32, tag="biasf")   # per-ktok exp bias

            # load bias row: token t = kt*128 + p
            bias_b = pbias.tile([P, TT], BF16, tag="biasb")
            nc.sync.dma_start(bias_b, bv(OBIAS, [[1, P], [P, TT]]))
            nc.vector.tensor_copy(bias_f, bias_b)

            # per-row weight dequant scales: the full scale vector is shipped
            # (replicated) on every core in column-major [col][p] order, so a
            # single strided DMA loads [P, 56] directly.
            FQ = FR // P  # wf2 f-tiles per chunk (4)
            NSC = 3 * KD + F // P  # 56 columns of 128 rows
            sc_cols = {"qkv": 0, "proj": KD, "f1": 2 * KD, "f2": 3 * KD}
            psc = pool("psc", 1)
            sc_b = psc.tile([P, NSC], BF16, tag="scb")
            nc.sync.dma_start(sc_b, bv(OSALL, [[1, P], [P, NSC]]))
            sc_f = psc.tile([P, NSC], FP32, tag="scf")
            nc.vector.tensor_copy(sc_f, sc_b)

            def sc_ap(name, idx):
                return sc_f[:, sc_cols[name] + idx : sc_cols[name] + idx + 1]

            # per-token x scales: all 4096 tokens in [P, TT] layout (token
            # t = kt*128 + p, same as the bias row), plus own 512 in [P, QT]
            sxa_b = psc.tile([P, TT], BF16, tag="sxab")
            nc.sync.dma_start(sxa_b, bv(OSXA, [[1, P], [P, TT]]))
            sxa_f = psc.tile([P, TT], FP32, tag="sxaf")
            nc.vector.tensor_copy(sxa_f, sxa_b)
            sxq_b = psc.tile([P, QT], BF16, tag="sxqb")
            nc.sync.dma_start(sxq_b, bv(OSX, [[1, P], [P, QT]]))
            sxq_f = psc.tile([P, QT], FP32, tag="sxqf")
            nc.vector.tensor_copy(sxq_f, sxq_b)

            ps_mm = pool("ps_mm", 3, space="PSUM")
            ps_tp = pool("ps_tp", 1, space="PSUM")
            ps_st = pool("ps_st", 1, space="PSUM")

            def norm_tile(px, xt, ptmp, pst):
                """xt [P, D] bf16 -> ht [P, D] bf16 (rmsnorm, gain folded in w)."""
                sq = ptmp.tile([P, D], BF16, tag="sq")
                ssq = pst.tile([P, 1], FP32, tag="ssq")
                nc.vector.tensor_mul(sq, xt, xt)
                nc.vector.tensor_reduce(ssq, sq, mybir.AxisListType.X, ALU.add)
                srt = pst.tile([P, 1], FP32, tag="srt")
                nc.scalar.activation(srt, ssq, AF.Sqrt, bias=eps_t, scale=1.0 / D)
                rstd = pst.tile([P, 1], FP32, tag="rstd")
                nc.vector.reciprocal(rstd, srt)
                ht = px.tile([P, D], BF16, tag="ht")
                nc.vector.tensor_scalar_mul(ht, xt, rstd)
                return ht

            def rope_window(ps, cs_src, prope, ptmp):
                """ps [P, HPW, DH] psum fp32 -> rop [P, W] bf16 (roped)."""
                csb = prope.tile([P, HPW, 2 * HALF], BF16, tag="csb")
                nc.sync.dma_start(csb, cs_src)
                csf = prope.tile([P, HPW, 2 * HALF], FP32, tag="csf")
                nc.vector.tensor_copy(csf, csb)
                crep = csf[:, :, 0:HALF]
                srep = csf[:, :, HALF : 2 * HALF]
                rop = ptmp.tile([P, W], BF16, tag="rop")
                rop3 = rop.rearrange("p (h j) -> p h j", j=DH)
                ta = prope.tile([P, HPW, HALF], BF16, tag="ta")
                tb = prope.tile([P, HPW, HALF], BF16, tag="tb")
                nc.vector.tensor_mul(ta, ps[:, :, 0:HALF], crep)
                nc.vector.tensor_mul(tb, ps[:, :, HALF:DH], srep)
                nc.vector.tensor_sub(rop3[:, :, 0:HALF], ta, tb)
                tc2 = prope.tile([P, HPW, HALF], BF16, tag="ta")
                td = prope.tile([P, HPW, HALF], BF16, tag="tb")
                nc.vector.tensor_mul(tc2, ps[:, :, HALF:DH], crep)
                nc.vector.tensor_mul(td, ps[:, :, 0:HALF], srep)
                nc.vector.tensor_add(rop3[:, :, HALF:DH], tc2, td)
                return rop

            with ExitStack() as c1:
                pool1 = lambda name, bufs, **kw: c1.enter_context(
                    tc.tile_pool(name=name, bufs=bufs, **kw)
                )
                pkT = pool1("pkT", 1)
                kT = pkT.tile([P, KD, T], BF16, tag="kT")     # roped k, [dh, hc, tok]
                pv = pool1("pv", 1)
                v65 = pv.tile([P, TT, H, DH + 1], BF16, tag="v65")
                nc.vector.memset(v65[:, :, :, DH : DH + 1], 1.0)
                ps_kv = pool1("ps_kv", 2, space="PSUM")

                # ---- K pass then V pass over all gathered tokens ----
                # each pass holds 2 weight windows (1024 cols) resident and
                # recomputes the hidden tile per 128-token tile.
                for vpass in range(2):  # 0: K cols, 1: V cols
                    with ExitStack() as c2:
                        pool2 = lambda name, bufs, **kw: c2.enter_context(
                            tc.tile_pool(name=name, bufs=bufs, **kw)
                        )
                        pw = pool2("pw", 1)
                        pxt = pool2("pxt", 1)
                        pht = pool2("pht", 2)
                        phid = pool2("phid", 2)
                        prope = pool2("prope", 2)
                        ptmp = pool2("ptmp", 1)
                        pst = pool2("pst", 2)
                        pw8 = pool2("pw8", 1)
                        wts = []
                        for wi in range(2):
                            w8 = pw8.tile([P, KD, W], I8, tag="w8")
                            off = R8QKV + (1 + vpass) * D + wi * W
                            nc.sync.dma_start(
                                w8,
                                gv8(off, [[3 * D, P], [W8SIZE, NCORES], [1, W]]),
                            )
                            wt = pw.tile([P, KD, W], BF16, tag=f"w{wi}")
                            for dc in range(KD):
                                nc.vector.tensor_scalar_mul(
                                    wt[:, dc, :], w8[:, dc, :], sc_ap("qkv", dc)
                                )
                            wts.append(wt)
                        for tt in range(TT):
                            ch, r0 = tt // 4, (tt % 4) * P
                            xt8 = pxt.tile([P, D], I8, tag="xt8")
                            nc.gpsimd.dma_start(
                                xt8,
                                gv8(ch * W8SIZE + R8X + r0 * D, [[D, P], [1, D]]),
                            )
                            xt = pxt.tile([P, D], BF16, tag="xt")
                            nc.vector.tensor_scalar_mul(
                                xt, xt8, sxa_f[:, tt : tt + 1]
                            )
                            ht = norm_tile(pht, xt, ptmp, pst)
                            hidt = phid.tile([P, KD, P], BF16, tag="hidt")
                            for c2i in range(KD):
                                tp = ps_tp.tile([P, P], BF16, tag="tpps")
                                nc.tensor.transpose(
                                    tp, ht[:, c2i * P : (c2i + 1) * P], ident
                                )
                                nc.vector.tensor_copy(hidt[:, c2i, :], tp)
                            for wi in range(2):
                                ps = ps_kv.tile([P, W], FP32, tag="kvps")
                                for dc in range(KD):
                                    nc.tensor.matmul(
                                        ps,
                                        hidt[:, dc, :],
                                        wts[wi][:, dc, :],
                                        start=(dc == 0),
                                        stop=(dc == KD - 1),
                                    )
                                ps3 = ps.rearrange("p (h j) -> p h j", j=DH)
                                if vpass == 1:
                                    h0 = wi * HPW
                                    nc.vector.tensor_copy(
                                        v65[:, tt, h0 : h0 + HPW, 0:DH], ps3
                                    )
                                else:
                                    # position rows (tt*128 % 2048) live in
                                    # gathered cs-shard chunk pos//SR
                                    pos = (tt * P) % S
                                    cs_src = gv(
                                        (pos // SR) * OW8 + OCS + (pos % SR) * 2 * HALF,
                                        [[2 * HALF, P], [0, HPW], [1, 2 * HALF]],
                                    )
                                    rop = rope_window(ps3, cs_src, prope, ptmp)
                                    for c2i in range(W // P):
                                        tp = ps_tp.tile([P, P], BF16, tag="tpps")
                                        nc.tensor.transpose(
                                            tp, rop[:, c2i * P : (c2i + 1) * P], ident
                                        )
                                        gc = wi * (W // P) + c2i
                                        nc.vector.tensor_copy(
                                            kT[:, gc, tt * P : (tt + 1) * P], tp
                                        )

                # ---- Q pass: own 512 tokens ----
                with ExitStack() as c2:
                    pool2 = lambda name, bufs, **kw: c2.enter_context(
                        tc.tile_pool(name=name, bufs=bufs, **kw)
                    )
                    phq = pool2("phq", 1)
                    hqT = phq.tile([P, KD, QN], BF16, tag="hqT")
                    pxt = pool2("pxt", 2)
                    pht = pool2("pht", 2)
                    prope = pool2("prope", 2)
                    ptmp = pool2("ptmp", 2)
                    pst = pool2("pst", 2)
                    pwq = pool2("pwq", 1)
                    for qt in range(QT):
                        xt8 = pxt.tile([P, D], I8, tag="xt8")
                        nc.gpsimd.dma_start(
                            xt8, bv8(R8X + qt * P * D, [[D, P], [1, D]])
                        )
                        xt = pxt.tile([P, D], BF16, tag="xt")
                        nc.vector.tensor_scalar_mul(
                            xt, xt8, sxq_f[:, qt : qt + 1]
                        )
                        # transpose own x into residual tile (fp32)
                        for c2i in range(KD):
                            tp = ps_tp.tile([P, P], BF16, tag="tpps")
                            nc.tensor.transpose(
                                tp, xt[:, c2i * P : (c2i + 1) * P], ident
                            )
                            nc.vector.tensor_copy(
                                xres[:, c2i, qt * P : (qt + 1) * P], tp
                            )
                        ht = norm_tile(pht, xt, ptmp, pst)
                        for c2i in range(KD):
                            tp = ps_tp.tile([P, P], BF16, tag="tpps")
                            nc.tensor.transpose(
                                tp, ht[:, c2i * P : (c2i + 1) * P], ident
                            )
                            nc.vector.tensor_copy(
                                hqT[:, c2i, qt * P : (qt + 1) * P], tp
                            )
                    pwq8 = pool2("pwq8", 1)
                    for wi in range(2):
                        w8 = pwq8.tile([P, KD, W], I8, tag="wq8")
                        nc.sync.dma_start(
                            w8,
                            gv8(
                                R8QKV + wi * W,
                                [[3 * D, P], [W8SIZE, NCORES], [1, W]],
                            ),
                        )
                        wt = pwq.tile([P, KD, W], BF16, tag="wq")
                        for dc in range(KD):
                            nc.vector.tensor_scalar_mul(
                                wt[:, dc, :], w8[:, dc, :], sc_ap("qkv", dc)
                            )
                        for qt in range(QT):
                            ps = ps_mm.tile([P, W], FP32, tag="mmps")
                            for dc in range(KD):
                                nc.tensor.matmul(
                                    ps,
                                    hqT[:, dc, qt * P : (qt + 1) * P],
                                    wt[:, dc, :],
                                    start=(dc == 0),
                                    stop=(dc == KD - 1),
                                )
                            ps3 = ps.rearrange("p (h j) -> p h j", j=DH)
                            cs_src = bv(
                                OCSQ + qt * P * 2 * HALF,
                                [[2 * HALF, P], [0, HPW], [1, 2 * HALF]],
                            )
                            rop = rope_window(ps3, cs_src, prope, ptmp)
                            for c2i in range(W // P):
                                tp = ps_tp.tile([P, P], BF16, tag="tpps")
                                nc.tensor.transpose(
                                    tp, rop[:, c2i * P : (c2i + 1) * P], ident
                                )
                                gc = wi * (W // P) + c2i
                                nc.vector.tensor_copy(
                                    qT[:, gc, qt * P : (qt + 1) * P], tp
                                )

                # ---- attention over all 4096 keys ----
                with ExitStack() as c2:
                    pool2 = lambda name, bufs, **kw: c2.enter_context(
                        tc.tile_pool(name=name, bufs=bufs, **kw)
                    )
                    pex = pool2("pex", 1)
                    phead = pool2("phead", 2)
                    for h in range(H):
                        hc, hp = h // 2, (h % 2) * DH
                        for qw in range(NQW):
                            qsl = qT[hp : hp + DH, hc, qw * QW : (qw + 1) * QW]
                            ex = pex.tile([P, TT, QW], BF16, tag="ex")
                            for kt in range(TT):
                                pss = ps_mm.tile([P, QW], FP32, tag="mmps")
                                nc.tensor.matmul(
                                    pss,
                                    kT[hp : hp + DH, hc, kt * P : (kt + 1) * P],
                                    qsl,
                                    start=True,
                                    stop=True,
                                )
                                nc.scalar.activation(
                                    ex[:, kt, :], pss, AF.Exp,
                                    bias=bias_f[:, kt : kt + 1],
                                    scale=1.0 / math.sqrt(DH),
                                )
                            pso = ps_mm.tile([DH + 1, QW], FP32, tag="mmps")
                            for kt in range(TT):
                                nc.tensor.matmul(
                                    pso,
                                    v65[:, kt, h, :],
                                    ex[:, kt, :],
                                    start=(kt == 0),
                                    stop=(kt == TT - 1),
                                )
                            rc = phead.tile([1, QW], FP32, tag="rcrow")
                            nc.vector.reciprocal(rc, pso[DH : DH + 1, :])
                            rb = ps_tp.tile([DH, QW], FP32, tag="tpps")
                            nc.tensor.matmul(
                                rb, ones_row[0:1, 0:DH], rc, start=True, stop=True
                            )
                            rbs = phead.tile([DH, QW], FP32, tag="rbsb")
                            nc.vector.tensor_copy(rbs, rb)
                            nc.vector.tensor_mul(
                                attn[hp : hp + DH, hc, qw * QW : (qw + 1) * QW],
                                pso[0:DH, :],
                                rbs,
                            )

            # ---- proj + residual (into xres in place) ----
            with ExitStack() as c1:
                pool1 = lambda name, bufs, **kw: c1.enter_context(
                    tc.tile_pool(name=name, bufs=bufs, **kw)
                )
                pwp = pool1("pwp", 2)
                pwp8 = pool1("pwp8", 2)
                for dt in range(KD):
                    wp8 = pwp8.tile([P, KD, P], I8, tag="wp8")
                    nc.sync.dma_start(
                        wp8,
                        gv8(R8PROJ + dt * P, [[D, P], [W8SIZE, NCORES], [1, P]]),
                    )
                    wp = pwp.tile([P, KD, P], BF16, tag="wp")
                    for ac in range(KD):
                        nc.vector.tensor_scalar_mul(
                            wp[:, ac, :], wp8[:, ac, :], sc_ap("proj", ac)
                        )
                    ps = ps_mm.tile([P, QN], FP32, tag="mmps")
                    for ac in range(KD):
                        nc.tensor.matmul(
                            ps, wp[:, ac, :], attn[:, ac, :],
                            start=(ac == 0), stop=(ac == KD - 1),
                        )
                    nc.vector.tensor_add(xres[:, dt, :], ps, xres[:, dt, :])

            # ---- norm2 + FFN ----
            with ExitStack() as c1:
                pool1 = lambda name, bufs, **kw: c1.enter_context(
                    tc.tile_pool(name=name, bufs=bufs, **kw)
                )
                psq2 = pool1("psq2", 2)
                prow = pool1("prow", 1)
                prstd = pool1("prstd", 1)
                ph2 = pool1("ph2", 1)
                st2 = ps_st.tile([1, QN], FP32, tag="stps")
                for dt in range(KD):
                    sq2 = psq2.tile([P, QN], BF16, tag="sq2")
                    nc.vector.tensor_mul(sq2, xres[:, dt, :], xres[:, dt, :])
                    nc.tensor.matmul(
                        st2, ones_col, sq2, start=(dt == 0), stop=(dt == KD - 1)
                    )
                rows2 = prow.tile([33, QN], FP32, tag="srow")
                nc.scalar.activation(
                    rows2[32:33, :], st2, AF.Sqrt, bias=eps_t[32:33], scale=1.0 / D
                )
                nc.vector.reciprocal(rows2[0:1, :], rows2[32:33, :])
                rstd2 = prstd.tile([P, QN], BF16, tag="rstd2")
                rb2 = ps_st.tile([P, QN], FP32, tag="stps")
                nc.tensor.matmul(rb2, ones_row, rows2[0:1, :], start=True, stop=True)
                nc.vector.tensor_copy(rstd2, rb2)
                h2 = ph2.tile([P, KD, QN], BF16, tag="h2")
                for dt in range(KD):
                    nc.vector.tensor_mul(h2[:, dt, :], xres[:, dt, :], rstd2)

                psil = pool1("psil", 1)
                pw1 = pool1("pw1", 2)
                ponat = pool1("ponat", 1)
                o_nat = ponat.tile([P, QT, D], BF16, tag="onat")
                sil = psil.tile([P, KF, QN], BF16, tag="sil")
                pw18 = pool1("pw18", 2)
                for ft in range(KF):
                    w18 = pw18.tile([P, KD, P], I8, tag="w18")
                    nc.sync.dma_start(
                        w18,
                        gv8(R8F1 + ft * P, [[F, P], [W8SIZE, NCORES], [1, P]]),
                    )
                    w1t = pw1.tile([P, KD, P], BF16, tag="w1t")
                    for dc in range(KD):
                        nc.vector.tensor_scalar_mul(
                            w1t[:, dc, :], w18[:, dc, :], sc_ap("f1", dc)
                        )
                    ps = ps_mm.tile([P, QN], FP32, tag="mmps")
                    for dc in range(KD):
                        nc.tensor.matmul(
                            ps, w1t[:, dc, :], h2[:, dc, :],
                            start=(dc == 0), stop=(dc == KD - 1),
                        )
                    nc.scalar.activation(sil[:, ft, :], ps, AF.Silu, bias=zero_t)
                pw2 = pool1("pw2", 2)
                pw28 = pool1("pw28", 2)
                pout = pool1("pout", 2)
                for dt in range(KD):
                    w28 = pw28.tile([P, NCORES, FQ, P], I8, tag="w28")
                    for cc in range(NCORES):
                        nc.sync.dma_start(
                            w28[:, cc, :, :],
                            gv8(
                                cc * W8SIZE + R8F2 + dt * P,
                                [[D, P], [P * D, FQ], [1, P]],
                            ),
                        )
                    w2t = pw2.tile([P, NCORES, FQ, P], BF16, tag="w2t")
                    for cc in range(NCORES):
                        for fq in range(FQ):
                            nc.vector.tensor_scalar_mul(
                                w2t[:, cc, fq, :],
                                w28[:, cc, fq, :],
                                sc_ap("f2", cc * FQ + fq),
                            )
                    ps = ps_mm.tile([P, QN], FP32, tag="mmps")
                    for fc in range(KF):
                        nc.tensor.matmul(
                            ps,
                            w2t[:, fc // FQ, fc % FQ, :],
                            sil[:, fc, :],
                            start=(fc == 0),
                            stop=(fc == KF - 1),
                        )
                    ot = pout.tile([P, QN], BF16, tag="outsb")
                    otf = pout.tile([P, QN], FP32, tag="outf")
                    nc.vector.tensor_add(otf, ps, xres[:, dt, :])
                    nc.vector.tensor_copy(ot, otf)
                    # transpose [D-chunk, tok] -> [tok, D-chunk]: natural layout
                    for qt in range(QT):
                        tp = ps_tp.tile([P, P], BF16, tag="tpps")
                        nc.tensor.transpose(tp, ot[:, qt * P : (qt + 1) * P], ident)
                        nc.vector.tensor_copy(
                            o_nat[:, qt, dt * P : (dt + 1) * P], tp
                        )
                # int8-quantize per token with a bf16 scale packed in the
                # last two bytes of each row
                omx = pout.tile([P, QT], FP32, tag="omx")
                nc.vector.tensor_reduce(
                    omx, o_nat, mybir.AxisListType.X, ALU.max,
                    apply_absolute_value=True,
                )
                oinv = pout.tile([P, QT], FP32, tag="oinv")
                nc.vector.reciprocal(oinv, omx)
                oinv2 = pout.tile([P, QT], FP32, tag="oinv2")
                nc.vector.tensor_scalar_mul(oinv2, oinv, 127.0)
                oscl = pout.tile([P, QT], BF16, tag="oscl")
                nc.vector.tensor_scalar_mul(oscl, omx, 1.0 / 127.0)
                oq = pout.tile([P, QT, D + 2], I8, tag="oq")
                for qt in range(QT):
                    nc.vector.tensor_scalar_mul(
                        oq[:, qt, 0:D], o_nat[:, qt, :], oinv2[:, qt : qt + 1]
                    )
                    nc.vector.tensor_copy(
                        oq[:, qt, D : D + 2], oscl[:, qt : qt + 1].bitcast(I8)
                    )
                    nc.sync.dma_start(
                        outd[qt * P : (qt + 1) * P, :], oq[:, qt, :]
                    )

    nc.finalize()
    return nc


def _rope_tables():
    inv = ROPE_BASE ** (-np.arange(HALF, dtype=np.float64) / HALF)
    fr = np.arange(S, dtype=np.float64)[:, None] * inv[None, :]
    cs = np.concatenate([np.cos(fr), np.sin(fr)], axis=1)
    return cs.astype(ml_dtypes.bfloat16)


def _quant_rows(w):
    """Per-row symmetric int8 quantization with bf16 scales."""
    bf = ml_dtypes.bfloat16
    s = (np.abs(w).max(axis=1) / 127.0).astype(bf)
    sf = s.astype(np.float32)
    sf[sf == 0] = 1.0
    q = np.rint(w / sf[:, None]).clip(-127, 127).astype(np.int8)
    return q, s


def make_in_maps(z_H, z_L, w_qkv, w_proj, w_ffn1, w_ffn2, g1, g2):
    bf = ml_dtypes.bfloat16
    x = np.asarray(z_H, np.float32) + np.asarray(z_L, np.float32)
    s_x = (np.abs(x).max(axis=-1) / 127.0).astype(bf)  # [B, S]
    s_xf = s_x.astype(np.float32)
    s_xf[s_xf == 0] = 1.0
    q_x = np.rint(x / s_xf[..., None]).clip(-127, 127).astype(np.int8)
    q_qkv, s_qkv = _quant_rows(np.asarray(g1, np.float32)[:, None] * np.asarray(w_qkv, np.float32))
    q_proj, s_proj = _quant_rows(np.asarray(w_proj, np.float32))
    q_f1, s_f1 = _quant_rows(np.asarray(g2, np.float32)[:, None] * np.asarray(w_ffn1, np.float32))
    q_f2, s_f2 = _quant_rows(np.asarray(w_ffn2, np.float32))
    cs = _rope_tables()
    in_maps, perms = [], []
    for c in range(NCORES):
        b, qo = c // CPB, (c % CPB) * QN
        blob = np.empty(BLOB, bf)
        b8 = blob.view(np.int8)
        blob[OCS : OCS + SR * DH] = cs[c * SR : (c + 1) * SR].ravel()
        blob[OCSQ : OCSQ + QN * DH] = cs[qo : qo + QN].ravel()
        bias = np.zeros(T, np.float32)
        other = slice(S, T) if b == 0 else slice(0, S)
        bias[other] = MASK_BIAS
        blob[OBIAS : OBIAS + T] = bias.astype(bf)
        blob[OSALL : OSALL + D] = s_qkv
        blob[OSALL + D : OSALL + 2 * D] = s_proj
        blob[OSALL + 2 * D : OSALL + 3 * D] = s_f1
        blob[OSALL + 3 * D : OSALL + 3 * D + F] = s_f2
        blob[OSX : OSX + QN] = s_x[b, qo : qo + QN]
        blob[OSXA : OSXA + T] = s_x.ravel()
        b8[BW + R8X : BW + R8X + QN * D] = q_x[b, qo : qo + QN].ravel()
        b8[BW + R8QKV : BW + R8QKV + P * 3 * D] = q_qkv[c * P : (c + 1) * P].ravel()
        b8[BW + R8PROJ : BW + R8PROJ + P * D] = q_proj[c * P : (c + 1) * P].ravel()
        b8[BW + R8F1 : BW + R8F1 + P * F] = q_f1[c * P : (c + 1) * P].ravel()
        b8[BW + R8F2 : BW + R8F2 + FR * D] = q_f2[c * FR : (c + 1) * FR].ravel()
        in_maps.append(dict(blob=blob))
        perms.append((b, qo))
    return in_maps, perms


_CACHED = {}


def kernel(z_H_previous, z_L_current, w_qkv, w_proj, w_ffn1, w_ffn2, g_norm1, g_norm2):
    assert z_H_previous.shape == (B, S, D)
    if "nc" not in _CACHED:
        nc = build_bass()
        # the program is immutable after finalize; memoize its BIR-json so the
        # per-call jit lowering doesn't re-serialize ~8 MB (~0.1 s) every run
        try:
            bir = nc.to_json_bytes()
            nc.to_json_bytes = lambda _b=bir: _b
        except Exception:
            pass
        _CACHED["nc"] = nc
    nc = _CACHED["nc"]
    in_maps, perms = make_in_maps(
        z_H_previous, z_L_current, w_qkv, w_proj, w_ffn1, w_ffn2, g_norm1, g_norm2
    )
    res = None
    for attempt in range(3):
        try:
            res = run_bass_kernel_spmd(nc, in_maps, core_ids=list(range(NCORES)))
            break
        except Exception:
            # transient device-unrecoverable states heal on backend re-init
            if attempt == 2:
                raise
            try:
                _jax.clear_backends()
            except Exception:
                pass
            import time as _time

            _time.sleep(3.0)
    out = np.empty((B, S, D), dtype=np.float32)
    for c in range(NCORES):
        b, qo = perms[c]
        oq = res.results[c]["outt"]  # [QN, D+2] int8
        scale = oq[:, D : D + 2].copy().view(ml_dtypes.bfloat16).astype(np.float32)
        out[b, qo : qo + QN, :] = oq[:, :D].astype(np.float32) * scale
    return out
